# revision 1
# baseline (speedup 1.0000x reference)
"""Compact bilinear pooling kernel for 8 Trainium2 NeuronCores.

Algorithm (host side folds everything into matmul weights):
  out[b,:,n] = circconv_1024(S1 @ x1[b,:,n], S2 @ x2[b,:,n])
Decomposed via x^1024-1 = (x^512-1)(x^512+1):
  cyclic-512 branch (rFFT512) + negacyclic-512 branch (odd DFT), both fused
  with the count-sketch matrices into dense real forward matrices
  W_j [512c -> 1024 freq rows], applied as f32r matmuls. Middle (complex
  multiply) runs on bf16 SBUF tiles on the vector engine. Inverse transforms
  are two block-diagonal [512 rows -> 512 outs] bf16 matmuls; the final
  unfold (c+d, c-d) happens on the vector engine reading inverse PSUM.

Sharding: batch 32 -> 4 per core (data parallel), weights replicated.
Layout: channels/freq rows on SBUF partitions, positions on free axis.
No transposes anywhere.
"""
import sys

sys.path.insert(0, "/opt/trn_rl_repo")

import numpy as np
import concourse.bass as bass
import concourse.mybir as mybir
from concourse import bacc
from concourse.tile import TileContext
from concourse.bass_utils import run_bass_kernel_spmd

B, C, HW, O = 32, 512, 784, 1024
NCORES = 8
BPC = B // NCORES  # 4 batches per core
PT = 392  # positions per tile (784 = 2*392; tiles never cross batch bounds)
NT = BPC * HW // PT  # 8 pos tiles per core
H = O // 2  # 512
F32, F32R, BF16 = mybir.dt.float32, mybir.dt.float32r, mybir.dt.bfloat16


def _build_host_matrices(sketch1, sketch2):
    """Fused fwd matrices [512 c, 1024 freq-rows] and inverse [512 rows, 512 out]."""

    def build_fwd(sketch):
        sk = np.asarray(sketch, dtype=np.float64)
        Sp = sk[:H] + sk[H:]
        Sm = sk[:H] - sk[H:]
        k = np.arange(H // 2 + 1)[:, None]
        n = np.arange(H)[None, :]
        Mc = np.exp(-2j * np.pi * k * n / H) @ Sp  # [257, C]
        ko = np.arange(H // 2)[:, None]
        Mo = np.exp(-2j * np.pi * n * (2 * ko + 1) / O) @ Sm  # [256, C]
        W = np.zeros((O, C))
        W[0:256] = Mc[0:256].real
        W[256] = Mc[256].real
        W[257:512] = Mc[1:256].imag
        W[512:768] = Mo.real
        W[768:1024] = Mo.imag
        return np.ascontiguousarray(W.T).astype(np.float32)  # [C, O]

    j = np.arange(H)[None, :]
    k = np.arange(256)[:, None]
    IC = np.zeros((H, H))
    IC[0:256] = 2 * np.cos(2 * np.pi * k * j / H) / H
    IC[0] = 1.0 / H
    IC[256] = np.cos(np.pi * j) / H
    ki = np.arange(1, 256)[:, None]
    IC[257:512] = -2 * np.sin(2 * np.pi * ki * j / H) / H
    ID = np.zeros((H, H))
    ID[0:256] = 2 * np.cos(2 * np.pi * (2 * k + 1) * j / O) / H
    ID[256:512] = -2 * np.sin(2 * np.pi * (2 * k + 1) * j / O) / H
    return (
        build_fwd(sketch1),
        build_fwd(sketch2),
        (IC / 2).astype(np.float32),
        (ID / 2).astype(np.float32),
    )


def _build_program():
    nc = bacc.Bacc(None)
    x1e = nc.declare_dram_parameter("x1", [BPC, C, HW], F32, isOutput=False)
    x2e = nc.declare_dram_parameter("x2", [BPC, C, HW], F32, isOutput=False)
    w1e = nc.declare_dram_parameter("w1", [C, O], F32, isOutput=False)
    w2e = nc.declare_dram_parameter("w2", [C, O], F32, isOutput=False)
    ice = nc.declare_dram_parameter("ic", [H, H], F32, isOutput=False)
    ide = nc.declare_dram_parameter("id", [H, H], F32, isOutput=False)
    oute = nc.declare_dram_parameter("out", [BPC, O, HW], F32, isOutput=True)

    with TileContext(nc) as tc:
        with (
            tc.tile_pool(name="wpool", bufs=1) as wpool,
            tc.tile_pool(name="xpool", bufs=2) as xpool,
            tc.tile_pool(name="fpool", bufs=2) as fpool,
            tc.tile_pool(name="opool", bufs=2) as opool,
            tc.tile_pool(name="psf", bufs=3, space="PSUM") as psf,
            tc.tile_pool(name="psc", bufs=2, space="PSUM") as psc,
            tc.tile_pool(name="psd", bufs=2, space="PSUM") as psd,
        ):
            # ---- weights: cast-DMA once (gpsimd SWDGE does dtype casts) ----
            w1r, w2r, ict, idt = [], [], [], []
            for cc in range(4):
                sl = slice(cc * 128, (cc + 1) * 128)
                t = wpool.tile([128, O], F32R, tag=f"w1r{cc}", name=f"w1r{cc}")
                nc.gpsimd.dma_start(out=t[:], in_=w1e[sl])
                w1r.append(t)
                t = wpool.tile([128, O], F32R, tag=f"w2r{cc}", name=f"w2r{cc}")
                nc.gpsimd.dma_start(out=t[:], in_=w2e[sl])
                w2r.append(t)
                t = wpool.tile([128, H], BF16, tag=f"ic{cc}", name=f"ic{cc}")
                nc.gpsimd.dma_start(out=t[:], in_=ice[sl])
                ict.append(t)
                t = wpool.tile([128, H], BF16, tag=f"id{cc}", name=f"id{cc}")
                nc.gpsimd.dma_start(out=t[:], in_=ide[sl])
                idt.append(t)

            # ---- main loop over position tiles ----
            for t in range(NT):
                b, n0 = t // 2, (t % 2) * PT
                nsl = slice(n0, n0 + PT)

                # x loads with cast f32 -> f32r
                xr = {}
                for j, xe in ((1, x1e), (2, x2e)):
                    for cc in range(4):
                        xt = xpool.tile(
                            [128, PT], F32R, tag=f"x{j}_{cc}", name=f"x{j}_{cc}_{t}"
                        )
                        nc.gpsimd.dma_start(
                            out=xt[:], in_=xe[b, cc * 128 : (cc + 1) * 128, nsl]
                        )
                        xr[(j, cc)] = xt

                # forward: fft_j[fc] [128 freq, PT] bf16
                fft = {}
                for j, wr in ((1, w1r), (2, w2r)):
                    for fc in range(8):
                        ps = psf.tile([128, PT], F32, tag="psf", name=f"psf{j}_{fc}_{t}")
                        for cc in range(4):
                            nc.tensor.matmul(
                                ps[:],
                                wr[cc][:, fc * 128 : (fc + 1) * 128],
                                xr[(j, cc)][:],
                                start=(cc == 0),
                                stop=(cc == 3),
                            )
                        ft = fpool.tile(
                            [128, PT], BF16, tag=f"fft{j}_{fc}", name=f"fft{j}_{fc}_{t}"
                        )
                        nc.scalar.copy(out=ft[:], in_=ps[:])
                        fft[(j, fc)] = ft

                # complex multiply on DVE (bf16): chunk pairs (re,im)
                prod = {}
                for re_c, im_c in ((0, 2), (1, 3), (4, 6), (5, 7)):
                    a1, b1 = fft[(1, re_c)], fft[(1, im_c)]
                    a2, b2 = fft[(2, re_c)], fft[(2, im_c)]
                    m1 = fpool.tile([128, PT], BF16, tag="m1", name=f"m1_{re_c}_{t}")
                    m2 = fpool.tile([128, PT], BF16, tag="m2", name=f"m2_{re_c}_{t}")
                    pr = fpool.tile(
                        [128, PT], BF16, tag=f"pr{re_c}", name=f"pr{re_c}_{t}"
                    )
                    pi = fpool.tile(
                        [128, PT], BF16, tag=f"pi{im_c}", name=f"pi{im_c}_{t}"
                    )
                    nc.vector.tensor_mul(m1[:], a1[:], a2[:])
                    nc.vector.tensor_mul(m2[:], b1[:], b2[:])
                    nc.vector.tensor_sub(pr[:], m1[:], m2[:])
                    nc.vector.tensor_mul(m1[:], a1[:], b2[:])
                    nc.vector.tensor_mul(m2[:], b1[:], a2[:])
                    nc.vector.tensor_add(pi[:], m1[:], m2[:])
                    if re_c == 0:
                        # row 0 of (0,2) pair: DC (re) and Nyquist (held in
                        # im slot row 0) are real-only products
                        nc.vector.tensor_mul(pr[0:1, :], a1[0:1, :], a2[0:1, :])
                        nc.vector.tensor_mul(pi[0:1, :], b1[0:1, :], b2[0:1, :])
                    prod[re_c] = pr
                    prod[im_c] = pi

                # inverse + unfold + store
                for oc in range(4):
                    osl = slice(oc * 128, (oc + 1) * 128)
                    pc = psc.tile([128, PT], F32, tag="psc", name=f"psc{oc}_{t}")
                    for rc in range(4):
                        nc.tensor.matmul(
                            pc[:],
                            ict[rc][:, osl],
                            prod[rc][:],
                            start=(rc == 0),
                            stop=(rc == 3),
                        )
                    cs = opool.tile([128, PT], F32, tag=f"cs{oc}", name=f"cs{oc}_{t}")
                    nc.scalar.copy(out=cs[:], in_=pc[:])
                    pd = psd.tile([128, PT], F32, tag="psd", name=f"psd{oc}_{t}")
                    for rc in range(4):
                        nc.tensor.matmul(
                            pd[:],
                            idt[rc][:, osl],
                            prod[4 + rc][:],
                            start=(rc == 0),
                            stop=(rc == 3),
                        )
                    lo = opool.tile([128, PT], F32, tag=f"lo{oc}", name=f"lo{oc}_{t}")
                    hi = opool.tile([128, PT], F32, tag=f"hi{oc}", name=f"hi{oc}_{t}")
                    nc.vector.tensor_add(lo[:], cs[:], pd[:])
                    nc.vector.tensor_sub(hi[:], cs[:], pd[:])
                    nc.sync.dma_start(out=oute[b, osl, nsl], in_=lo[:])
                    nc.sync.dma_start(
                        out=oute[b, slice(512 + oc * 128, 512 + (oc + 1) * 128), nsl],
                        in_=hi[:],
                    )

    nc.finalize()
    return nc


_NC_CACHE = None


def kernel(x1, x2, sketch1, sketch2):
    global _NC_CACHE
    w1, w2, ic, idm = _build_host_matrices(sketch1, sketch2)
    if _NC_CACHE is None:
        _NC_CACHE = _build_program()
    nc = _NC_CACHE
    x1f = np.ascontiguousarray(np.asarray(x1, dtype=np.float32).reshape(B, C, HW))
    x2f = np.ascontiguousarray(np.asarray(x2, dtype=np.float32).reshape(B, C, HW))
    in_maps = []
    for i in range(NCORES):
        bs = slice(i * BPC, (i + 1) * BPC)
        in_maps.append(
            {
                "x1": np.ascontiguousarray(x1f[bs]),
                "x2": np.ascontiguousarray(x2f[bs]),
                "w1": w1,
                "w2": w2,
                "ic": ic,
                "id": idm,
            }
        )
    res = run_bass_kernel_spmd(nc, in_maps, list(range(NCORES)))
    out = np.concatenate([res.results[i]["out"] for i in range(NCORES)], axis=0)
    return out.reshape(B, O, 28, 28).astype(np.float32)


if __name__ == "__main__":
    rng = np.random.default_rng(0)
    x1 = rng.standard_normal((B, C, 28, 28)).astype(np.float32)
    x2 = rng.standard_normal((B, C, 28, 28)).astype(np.float32)
    h1 = rng.integers(0, O, C)
    s1 = rng.integers(0, 2, C) * 2.0 - 1.0
    h2 = rng.integers(0, O, C)
    s2 = rng.integers(0, 2, C) * 2.0 - 1.0
    sk1 = np.zeros((O, C), np.float32)
    sk1[h1, np.arange(C)] = s1
    sk2 = np.zeros((O, C), np.float32)
    sk2[h2, np.arange(C)] = s2
    got = kernel(x1, x2, sk1, sk2)
    p1 = np.einsum("bchw,oc->bohw", x1, sk1).reshape(B, O, HW)
    p2 = np.einsum("bchw,oc->bohw", x2, sk2).reshape(B, O, HW)
    ref = np.fft.ifft(np.fft.fft(p1, axis=1) * np.fft.fft(p2, axis=1), axis=1).real
    err = np.abs(got.reshape(B, O, HW) - ref).max() / np.abs(ref).max()
    print("self-test max rel err:", err)
